# revision 22
# baseline (speedup 1.0000x reference)
"""Trainium2 Bass kernel for nn_DPSV_Loss (YOLOX-style detection loss).

Strategy
--------
Data parallel over batch: B=32 images sharded 4-per-core across 8 NeuronCores.

Key algorithmic insight: the center-radius assignment uses r = 0.5*stride, so
each ground-truth box matches EXACTLY ONE grid cell per FPN scale (the cell
containing its center), or none when the center sits exactly on a cell
boundary (strict inequalities).  First-match (argmax) semantics reduce to
"earliest gt wins each cell", resolvable on a tiny [G, G] conflict matrix.
The dense [B, G, A] candidate tensor of the reference never needs to exist.

Per core the device program:
  1. computes, for each (batch, scale, gt), the matched cell + validity,
  2. resolves first-match winners via a [50, 12*64] equality/conflict matrix
     (column-summed on the TensorEngine),
  3. gathers the 21 input channels at winner cells (ap_gather from SBUF),
  4. evaluates the vec/obj/cls loss terms on the ~150 foreground columns,
  5. densely evaluates only the objectness background term log(1-obj+eps)
     over all anchors (the single truly dense reduction in the loss),
  6. writes 7 partial sums; the host combines partials from the 8 cores.
"""

import sys

_TRN_REPO = "/opt/trn_rl_repo"
if _TRN_REPO not in sys.path:
    sys.path.insert(0, _TRN_REPO)

import numpy as np

import concourse.bass as bass
import concourse.bacc as bacc
import concourse.tile as tile
from concourse import mybir
from concourse.bass_utils import run_bass_kernel_spmd
from concourse.masks import make_identity

F32 = mybir.dt.float32
I32 = mybir.dt.int32
I16 = mybir.dt.int16
OP = mybir.AluOpType
AF = mybir.ActivationFunctionType
AX = mybir.AxisListType

NCORES = 8
B = 32
BL = B // NCORES          # batches per core
G = 50                    # ground truths per image
C = 16                    # num classes
CH = 5 + C                # channels (x, y, w, h, obj, cls...)
STRIDES = (8, 16, 32)
WS = (128, 64, 32)
HWS = (128 * 128, 64 * 64, 32 * 32)
NB = BL * 3               # (batch, scale) blocks; partition p = b*3 + s
BLK = 64                  # columns reserved per block (50 real + 14 pad)
NCOL = NB * BLK           # 768
EPS = 1e-8
INPUT_W = 1024.0


def _build_program():
    nc = bacc.Bacc("TRN2", target_bir_lowering=False)

    inp_d = [
        nc.dram_tensor("input0", [BL, CH, 128, 128], F32, kind="ExternalInput"),
        nc.dram_tensor("input1", [BL, CH, 64, 64], F32, kind="ExternalInput"),
        nc.dram_tensor("input2", [BL, CH, 32, 32], F32, kind="ExternalInput"),
    ]
    labels_d = nc.dram_tensor("labels", [BL, G, 5], F32, kind="ExternalInput")
    # dump index (HW_s) per column
    crow_d = nc.dram_tensor("crow", [1, NCOL], F32, kind="ExternalInput")
    # cw4: vec-loss weights [s/1024, s/1024, 10, 10] per column
    cw4_d = nc.dram_tensor("cw4", [4, NCOL], F32, kind="ExternalInput")
    # cst[:, 0] = 1/stride, cst[:, 1] = W per (b, s) partition
    cst_d = nc.dram_tensor("cst", [NB, 2], F32, kind="ExternalInput")
    part_d = nc.dram_tensor("partials", [8], F32, kind="ExternalOutput")

    with tile.TileContext(nc) as tc:
        _body(tc, inp_d, labels_d, crow_d, cw4_d, cst_d, part_d)
    if not nc.is_finalized():
        nc.finalize()
    return nc


def _body(tc, inp_d, labels_d, crow_d, cw4_d, cst_d, part_d):
    nc = tc.nc
    from contextlib import ExitStack

    ctx = ExitStack()
    sg = ctx.enter_context(tc.tile_pool(name="sg", bufs=1))
    inp_pool = ctx.enter_context(tc.tile_pool(name="inp", bufs=1))
    psum = ctx.enter_context(tc.tile_pool(name="psum", bufs=1, space="PSUM"))

    # NOTE: every compute-engine access must start at partition 0/32/64/96
    # (BIR verifier checkLegalPartitionAccess).  DMAs are exempt, so row
    # extraction/packing between differently-based tiles goes through DMA,
    # and partition broadcasts/reductions go through the TensorEngine.

    # ---------------- constants ----------------
    crow = sg.tile([1, NCOL], F32)   # dump idx (HW_s) per column
    nc.sync.dma_start(crow[:], crow_d[:])
    CW4 = sg.tile([4, NCOL], F32)    # vec weights: [s/1024, s/1024, 10, 10]
    nc.sync.dma_start(CW4[:], cw4_d[:])
    cst = sg.tile([NB, 2], F32)
    nc.sync.dma_start(cst[:], cst_d[:])

    ident = sg.tile([NB, NB], F32)
    make_identity(nc, ident[:])

    ones50 = sg.tile([G, 1], F32)
    nc.vector.memset(ones50[:], 1.0)
    ones16 = sg.tile([C, 1], F32)
    nc.vector.memset(ones16[:], 1.0)
    ones128 = sg.tile([128, 1], F32)
    nc.vector.memset(ones128[:], 1.0)
    onesr2 = sg.tile([1, 2], F32)
    nc.vector.memset(onesr2[:], 1.0)
    onesr4 = sg.tile([1, 4], F32)
    nc.vector.memset(onesr4[:], 1.0)
    onesr16 = sg.tile([1, C], F32)
    nc.vector.memset(onesr16[:], 1.0)
    onesr50 = sg.tile([1, G], F32)
    nc.vector.memset(onesr50[:], 1.0)

    epsb1 = sg.tile([1, 1], F32)
    nc.vector.memset(epsb1[:], EPS)
    epsb128 = sg.tile([128, 1], F32)
    nc.vector.memset(epsb128[:], EPS)

    cvals_i = sg.tile([C, 1], I32)
    nc.gpsimd.iota(cvals_i[:], pattern=[[0, 1]], base=0, channel_multiplier=1)
    cvals = sg.tile([C, 1], F32)
    nc.vector.tensor_copy(cvals[:], cvals_i[:])

    # lowm[g', g] = 1.0 if g' < g else 0.0   [G, BLK]
    lowm_i = sg.tile([G, BLK], I32)
    nc.gpsimd.iota(lowm_i[:], pattern=[[-1, BLK]], base=0, channel_multiplier=1)
    lowm_f = sg.tile([G, BLK], F32)
    nc.vector.tensor_copy(lowm_f[:], lowm_i[:])
    lowm = sg.tile([G, BLK], F32)
    nc.vector.tensor_single_scalar(lowm[:], lowm_f[:], 0.0, OP.is_lt)

    # ---------------- labels ----------------
    lab34 = sg.tile([2, BL, G], F32)     # w, h targets
    nc.sync.dma_start(lab34[:], labels_d[:, :, 2:4].rearrange("b g c -> c b g"))
    labC = sg.tile([1, BL, G], F32)      # class ids
    nc.sync.dma_start(labC[:], labels_d[:, :, 4:5].rearrange("b g c -> c b g"))

    # XY[p=(b*3+s), g, c] = labels[b, g, c] for c in (x, y)
    XY = sg.tile([NB, G, 2], F32)
    xy_src = labels_d[:, :, 0:2]  # [BL, G, 2]
    for s in range(3):
        out_ap = XY[:].rearrange("(b s) g c -> b s g c", s=3)[:, s, :, :]
        nc.sync.dma_start(out_ap, xy_src)

    # ---------------- stage A: cells & validity  [NB, G, 2] ----------------
    U = sg.tile([NB, G, 2], F32)
    nc.vector.tensor_scalar(U[:], XY[:], cst[:, 0:1], None, op0=OP.mult)  # xy/stride
    T1 = sg.tile([NB, G, 2], F32)
    nc.vector.tensor_single_scalar(T1[:], U[:], 0.5, OP.subtract)
    AXi = sg.tile([NB, G, 2], I32)
    nc.vector.tensor_copy(AXi[:], T1[:])          # ~floor (any rounding, +-1 ok)
    AXf = sg.tile([NB, G, 2], F32)
    nc.vector.tensor_copy(AXf[:], AXi[:])
    # fix: ax = ax0 + (u >= ax0+1) - (u < ax0)
    T2 = sg.tile([NB, G, 2], F32)
    nc.vector.tensor_single_scalar(T1[:], AXf[:], 1.0, OP.add)      # ax0+1
    nc.vector.tensor_tensor(T2[:], U[:], T1[:], OP.is_ge)
    nc.vector.tensor_tensor(AXf[:], AXf[:], T2[:], OP.add)
    nc.vector.tensor_tensor(T2[:], U[:], AXf[:], OP.is_lt)
    nc.vector.tensor_tensor(AXf[:], AXf[:], T2[:], OP.subtract)

    # in-bounds (strict): ax < u < ax+1
    nc.vector.tensor_single_scalar(T1[:], AXf[:], 1.0, OP.add)
    nc.vector.tensor_tensor(T2[:], U[:], AXf[:], OP.is_gt)
    INB = sg.tile([NB, G, 2], F32)
    nc.vector.tensor_tensor(INB[:], U[:], T1[:], OP.is_lt)
    nc.vector.tensor_tensor(INB[:], INB[:], T2[:], OP.mult)

    valid = sg.tile([NB, G], F32)
    nc.vector.tensor_tensor(valid[:], INB[:, :, 0], INB[:, :, 1], OP.mult)

    UAX = sg.tile([NB, G, 2], F32)  # u - ax  (fractional offset, decode units)
    nc.vector.tensor_tensor(UAX[:], U[:], AXf[:], OP.subtract)
    U0 = sg.tile([NB, G], F32)
    nc.vector.tensor_copy(U0[:], UAX[:, :, 0])
    U1 = sg.tile([NB, G], F32)
    nc.vector.tensor_copy(U1[:], UAX[:, :, 1])

    cell = sg.tile([NB, G], F32)    # ay*W + ax
    nc.vector.tensor_scalar(cell[:], AXf[:, :, 1], cst[:, 1:2], None, op0=OP.mult)
    nc.vector.tensor_tensor(cell[:], cell[:], AXf[:, :, 0], OP.add)

    # ---------------- reshape to row layout [*, NCOL] (via DMA) ----------------
    # Each [1, NCOL] row is staged as a zero-padded [12, 64] tile built
    # entirely on DVE (single producer semaphore), then flattened by one
    # full-width contiguous DMA (keeps per-instruction wait counts tiny).
    def pad64(src_ap, p):
        t64 = sg.tile([p, NB, BLK], F32, name=f"pad64_{nc.next_id()}")
        nc.vector.memset(t64[:], 0.0)
        nc.vector.tensor_copy(t64[:, :, 0:G] if p == 1 else t64[:, :, 0:G], src_ap)
        return t64

    valid64 = sg.tile([NB, BLK], F32)
    nc.vector.memset(valid64[:], 0.0)
    nc.vector.tensor_copy(valid64[:, 0:G], valid[:])
    cell64 = sg.tile([NB, BLK], F32)
    nc.vector.memset(cell64[:], 0.0)
    nc.vector.tensor_copy(cell64[:, 0:G], cell[:])
    u064 = sg.tile([NB, BLK], F32)
    nc.vector.memset(u064[:], 0.0)
    nc.vector.tensor_copy(u064[:, 0:G], U0[:])
    u164 = sg.tile([NB, BLK], F32)
    nc.vector.memset(u164[:], 0.0)
    nc.vector.tensor_copy(u164[:, 0:G], U1[:])
    tv64 = sg.tile([2, BL, 3, BLK], F32)
    nc.vector.memset(tv64[:], 0.0)
    nc.vector.tensor_copy(
        tv64[:, :, :, 0:G], lab34[:].unsqueeze(2).broadcast_to([2, BL, 3, G])
    )
    cls64 = sg.tile([1, BL, 3, BLK], F32)
    nc.vector.memset(cls64[:], 0.0)
    nc.vector.tensor_copy(
        cls64[:, :, :, 0:G], labC[:].unsqueeze(2).broadcast_to([1, BL, 3, G])
    )

    VALIDR = sg.tile([1, NCOL], F32)
    nc.sync.dma_start(VALIDR[:], valid64[:])
    CELLR = sg.tile([1, NCOL], F32)
    nc.sync.dma_start(CELLR[:], cell64[:])
    # TGT4 rows: u0, u1, t_w, t_h  (prescaled by CW4 later)
    TGT4 = sg.tile([4, NCOL], F32)
    nc.sync.dma_start(TGT4[0:1], u064[:])
    nc.sync.dma_start(TGT4[1:2], u164[:])
    nc.sync.dma_start(TGT4[2:4], tv64[:].rearrange("p b s k -> p (b s k)"))
    CLSR = sg.tile([1, NCOL], F32)
    nc.sync.dma_start(CLSR[:], cls64[:].rearrange("p b s k -> p (b s k)"))

    # ---------------- transposes (PE): cell/valid -> [G, NB] ----------------
    cvTp = psum.tile([G, NB], F32)
    nc.tensor.transpose(cvTp[:], cell[:], ident[:])
    vvTp = psum.tile([G, NB], F32)
    nc.tensor.transpose(vvTp[:], valid[:], ident[:])
    cvT = sg.tile([G, 2, NB], F32)
    nc.vector.tensor_copy(cvT[:, 0, :], cvTp[:])
    nc.vector.tensor_copy(cvT[:, 1, :], vvTp[:])

    # ---------------- E-stage: first-match winners ----------------
    # broadcast cell row across G partitions via rank-1 matmul
    crow_ps = psum.tile([G, NCOL], F32, tag="wide", bufs=2)
    for lo in range(0, NCOL, 512):
        hi = min(lo + 512, NCOL)
        nc.tensor.matmul(crow_ps[:, lo:hi], lhsT=onesr50[:], rhs=CELLR[0:1, lo:hi],
                         start=True, stop=True)

    # E2[g', (bs, g)] = valid[bs, g'] * (cell[bs, g'] == cell[bs, g]) * (g' < g)
    E2 = sg.tile([G, NB, BLK], F32)
    cellT_b = cvT[:, 0, :].unsqueeze(2).broadcast_to([G, NB, BLK])
    validT_b = cvT[:, 1, :].unsqueeze(2).broadcast_to([G, NB, BLK])
    cell_row = crow_ps[:].rearrange("g (n k) -> g n k", k=BLK)
    lowm_b = lowm[:].unsqueeze(1).broadcast_to([G, NB, BLK])
    nc.vector.tensor_tensor(E2[:], cellT_b, cell_row, OP.is_equal)
    nc.vector.tensor_tensor(E2[:], E2[:], validT_b, OP.mult)
    nc.vector.tensor_tensor(E2[:], E2[:], lowm_b, OP.mult)

    # conflict[j] = sum_g' E2[g', j]  (TensorEngine column sum)
    E2f = E2[:].rearrange("g n k -> g (n k)")
    cpsum = psum.tile([1, NCOL], F32, tag="wide", bufs=2)
    for lo in range(0, NCOL, 512):
        hi = min(lo + 512, NCOL)
        nc.tensor.matmul(cpsum[:, lo:hi], lhsT=ones50[:], rhs=E2f[:, lo:hi],
                         start=True, stop=True)

    # win = valid & (conflict == 0)
    WINR = sg.tile([1, NCOL], F32)
    nc.vector.tensor_single_scalar(WINR[:], cpsum[:], 0.0, OP.is_equal)
    nc.vector.tensor_tensor(WINR[:], WINR[:], VALIDR[0:1], OP.mult)

    # ---------------- gather indices ----------------
    SCR1 = sg.tile([1, NCOL], F32)
    nc.vector.tensor_tensor(SCR1[:], CELLR[0:1], crow[0:1], OP.subtract)
    nc.vector.tensor_tensor(SCR1[:], SCR1[:], WINR[:], OP.mult)
    nc.vector.tensor_tensor(SCR1[:], SCR1[:], crow[0:1], OP.add)
    idx16 = sg.tile([1, NCOL], I16)
    nc.vector.tensor_copy(idx16[:], SCR1[:])
    idxw = sg.tile([32, NCOL // 16], I16)
    idx_dram = nc.dram_tensor("idx_scratch", [NCOL], I16)
    nc.sync.dma_start(idx_dram[:], idx16[:])
    nc.sync.dma_start(idxw[0:16], idx_dram[:].rearrange("(k a) -> a k", a=16))
    nc.sync.dma_start(idxw[16:32], idx_dram[:].rearrange("(k a) -> a k", a=16))

    # ---------------- stream inputs, gather winner columns ----------------
    GT32 = sg.tile([32, NCOL], F32)
    for b in range(BL):
        for s in range(3):
            hw = HWS[s]
            t = inp_pool.tile(
                [32, hw + 1], F32, tag=f"s{s}", name=f"in_b{b}s{s}",
                bufs=2 if s > 0 else 1,
            )
            nc.sync.dma_start(
                t[0:CH, 0:hw], inp_d[s][b].rearrange("c h w -> c (h w)")
            )
            nc.gpsimd.memset(t[:, hw : hw + 1], 0.0)  # dump column
            blk = b * 3 + s
            idxs = idxw[:, blk * (BLK // 16) : (blk + 1) * (BLK // 16)]
            nc.gpsimd.ap_gather(
                out_ap=GT32[:, blk * BLK : (blk + 1) * BLK],
                in_ap=t[:],
                idxs_ap=idxs,
                channels=32,
                num_elems=hw + 1,
                d=1,
                num_idxs=BLK,
            )

    # ---------------- split gathered rows into aligned tiles (DMA) ----------
    OBJR = sg.tile([1, NCOL], F32)
    nc.sync.dma_start(OBJR[:], GT32[4:5])
    CLS16 = sg.tile([C, NCOL], F32)
    nc.sync.dma_start(CLS16[:], GT32[5:21])

    # ---------------- vec loss ----------------
    # D = TGT4*CW4 - GT32[0:4]*CW4 ; masked by win (rank-1 matmul broadcast)
    nc.vector.tensor_tensor(TGT4[:], TGT4[:], CW4[:], OP.mult)
    nc.vector.tensor_tensor(GT32[0:4], GT32[0:4], CW4[:], OP.mult)
    nc.vector.tensor_tensor(TGT4[:], TGT4[:], GT32[0:4], OP.subtract)
    win4_ps = psum.tile([4, NCOL], F32, tag="wide", bufs=2)
    for lo in range(0, NCOL, 512):
        hi = min(lo + 512, NCOL)
        nc.tensor.matmul(win4_ps[:, lo:hi], lhsT=onesr4[:], rhs=WINR[0:1, lo:hi],
                         start=True, stop=True)
    nc.vector.tensor_tensor(TGT4[:], TGT4[:], win4_ps[:], OP.mult)
    avec = sg.tile([4, 1], F32)
    nc.vector.tensor_reduce(avec[:], TGT4[:], axis=AX.X, op=OP.add,
                            apply_absolute_value=True)

    # ---------------- obj foreground part ----------------
    OT = sg.tile([1, NCOL], F32)
    nc.vector.tensor_scalar(OT[:], OBJR[:], -1.0, 1.0, op0=OP.mult, op1=OP.add)
    nc.scalar.activation(OBJR[:], OBJR[:], AF.Ln, bias=epsb1[:, 0:1], scale=1.0)
    nc.scalar.activation(OT[:], OT[:], AF.Ln, bias=epsb1[:, 0:1], scale=1.0)
    objacc = sg.tile([1, 1], F32)
    nc.vector.scalar_tensor_tensor(
        SCR1[:], OBJR[:], 1000.0, OT[:], op0=OP.mult, op1=OP.subtract
    )
    nc.vector.scalar_tensor_tensor(
        OT[:], SCR1[:], 1.0, WINR[:], op0=OP.mult, op1=OP.mult,
        accum_out=objacc[:],
    )

    # ---------------- cls loss ----------------
    T16 = sg.tile([C, NCOL], F32)
    nc.vector.tensor_scalar(T16[:], CLS16[:], -1.0, 1.0, op0=OP.mult, op1=OP.add)
    LP16 = sg.tile([C, NCOL], F32)
    nc.scalar.activation(LP16[:], CLS16[:], AF.Ln, bias=0.0, scale=1.0)
    nc.scalar.activation(T16[:], T16[:], AF.Ln, bias=0.0, scale=1.0)     # lq16
    nc.vector.tensor_single_scalar(LP16[:], LP16[:], -100.0, OP.max)
    nc.vector.tensor_single_scalar(T16[:], T16[:], -100.0, OP.max)
    # ct one-hot: broadcast cls row via rank-1 matmul, compare to per-part c
    clsb_ps = psum.tile([C, NCOL], F32, tag="wide", bufs=2)
    for lo in range(0, NCOL, 512):
        hi = min(lo + 512, NCOL)
        nc.tensor.matmul(clsb_ps[:, lo:hi], lhsT=onesr16[:], rhs=CLSR[0:1, lo:hi],
                         start=True, stop=True)
    nc.vector.tensor_scalar(CLS16[:], clsb_ps[:], cvals[:, 0:1], None,
                            op0=OP.is_equal)                              # ct
    nc.vector.tensor_tensor(LP16[:], LP16[:], T16[:], OP.subtract)        # d
    nc.vector.tensor_tensor(CLS16[:], CLS16[:], LP16[:], OP.mult)         # ct*d
    nc.vector.tensor_tensor(CLS16[:], CLS16[:], T16[:], OP.add)           # e
    epsum = psum.tile([1, NCOL], F32, tag="wide", bufs=2)
    for lo in range(0, NCOL, 512):
        hi = min(lo + 512, NCOL)
        nc.tensor.matmul(epsum[:, lo:hi], lhsT=ones16[:], rhs=CLS16[:, lo:hi],
                         start=True, stop=True)
    clsacc = sg.tile([1, 1], F32)
    nc.vector.scalar_tensor_tensor(
        SCR1[:], epsum[:], 1.0, WINR[:], op0=OP.mult, op1=OP.mult,
        accum_out=clsacc[:],
    )

    # ---------------- dense obj background term ----------------
    OBJ = sg.tile([128, BL, 168], F32)
    col0 = 0
    for s in range(3):
        k = HWS[s] // 128
        src = inp_d[s][:, 4].rearrange("b h w -> b (h w)").rearrange(
            "b (p j) -> p b j", p=128
        )
        nc.sync.dma_start(OBJ[:, :, col0 : col0 + k], src)
        col0 += k
    OBJf = OBJ[:].rearrange("p b j -> p (b j)")
    nc.vector.tensor_scalar(OBJf, OBJf, -1.0, 1.0, op0=OP.mult, op1=OP.add)
    dacc = sg.tile([128, 1], F32)
    nc.scalar.activation(OBJf, OBJf, AF.Ln, bias=epsb128[:, 0:1], scale=1.0,
                         accum_out=dacc[:])
    dsum_ps = psum.tile([1, 1], F32)
    nc.tensor.matmul(dsum_ps[:], lhsT=dacc[:], rhs=ones128[:],
                     start=True, stop=True)
    dsum = sg.tile([1, 1], F32)
    nc.vector.tensor_copy(dsum[:], dsum_ps[:])

    # ---------------- outputs ----------------
    nc.sync.dma_start(part_d[0:4], avec[:])
    nc.sync.dma_start(part_d[4:5], objacc[:])
    nc.sync.dma_start(part_d[5:6], dsum[:])
    nc.sync.dma_start(part_d[6:7], clsacc[:])

    ctx.close()


_PROG = None
TRACE = False
LAST_RESULTS = None


def _get_program():
    global _PROG
    if _PROG is None:
        _PROG = _build_program()
    return _PROG


def _host_constants():
    w0 = np.zeros((NCOL,), np.float32)
    dump = np.zeros((NCOL,), np.float32)
    for b in range(BL):
        for s in range(3):
            blk = b * 3 + s
            w0[blk * BLK : (blk + 1) * BLK] = STRIDES[s] / INPUT_W
            dump[blk * BLK : (blk + 1) * BLK] = HWS[s]
    crow = dump[None, :]
    cw4 = np.stack([w0, w0, np.full_like(w0, 10.0), np.full_like(w0, 10.0)])
    cst = np.zeros((NB, 2), np.float32)
    for b in range(BL):
        for s in range(3):
            cst[b * 3 + s, 0] = 1.0 / STRIDES[s]
            cst[b * 3 + s, 1] = WS[s]
    return crow, cw4, cst


def kernel(input0, input1, input2, labels):
    nc = _get_program()
    crow, cw4, cst = _host_constants()
    in_maps = []
    for c in range(NCORES):
        sl = slice(c * BL, (c + 1) * BL)
        in_maps.append(
            {
                "input0": np.ascontiguousarray(input0[sl], np.float32),
                "input1": np.ascontiguousarray(input1[sl], np.float32),
                "input2": np.ascontiguousarray(input2[sl], np.float32),
                "labels": np.ascontiguousarray(labels[sl], np.float32),
                "crow": crow,
                "cw4": cw4,
                "cst": cst,
            }
        )
    global LAST_RESULTS
    res = run_bass_kernel_spmd(
        nc, in_maps, core_ids=list(range(NCORES)), trace=TRACE
    )
    LAST_RESULTS = res
    P = np.stack([r["partials"] for r in res.results]).astype(np.float64)
    a_sum = P[:, 0:4].sum(axis=0)
    loss_obj = -(P[:, 4].sum() + P[:, 5].sum())
    loss_cls = -500.0 * P[:, 6].sum()
    loss_vec = a_sum.sum() * 10000.0
    loss = np.float32(loss_obj + loss_vec + loss_cls)
    return (np.array(loss, np.float32), a_sum.astype(np.float32))


if __name__ == "__main__":
    # smoke build
    _get_program()
    print("program built OK")


# revision 24
# speedup vs baseline: 2.2637x; 2.2637x over previous
"""Trainium2 Bass kernel for nn_DPSV_Loss (YOLOX-style detection loss).

Strategy
--------
Data parallel over batch: B=32 images sharded 4-per-core across 8 NeuronCores.

Key algorithmic insight: the center-radius assignment uses r = 0.5*stride, so
each ground-truth box matches EXACTLY ONE grid cell per FPN scale (the cell
containing its center), or none when the center sits exactly on a cell
boundary (strict inequalities).  First-match (argmax over gts) semantics
reduce to "earliest valid gt wins each cell", resolvable on a tiny conflict
matrix over gt pairs.  The dense [B, G, A] candidate tensor of the reference
never materializes, and of the 21*A inputs per image only the objectness
channel is ever needed densely — everything else is consumed at the <=150
foreground anchors per image.

Per core the device program:
  1. computes cell/validity per (batch, scale, gt)          [12, 50] tiles
  2. resolves first-match winners via an equality/conflict matrix
     column-summed on the TensorEngine                      [50, 768]
  3. gathers the 21 channels at winner anchors straight from DRAM
     (channels-last staging => one 84B contiguous row per winner)
  4. evaluates vec/obj/cls loss terms winner-major          [128, 6, *]
  5. densely reduces only log(1-obj+eps) over all anchors   [128, 672]
  6. emits 7 partial sums via one [128, 8] x ones matmul; host combines
     the 8 cores.

The host-side kernel() only re-lays-out inputs (transpose to channels-last +
a contiguous copy of the obj channel) and shards the batch; all arithmetic
happens on device.
"""

import sys

_TRN_REPO = "/opt/trn_rl_repo"
if _TRN_REPO not in sys.path:
    sys.path.insert(0, _TRN_REPO)

import numpy as np

import concourse.bass as bass
import concourse.bacc as bacc
import concourse.tile as tile
from concourse import mybir
from concourse.bass_utils import run_bass_kernel_spmd
from concourse.masks import make_identity

F32 = mybir.dt.float32
I32 = mybir.dt.int32
OP = mybir.AluOpType
AF = mybir.ActivationFunctionType
AX = mybir.AxisListType

NCORES = 8
B = 32
BL = B // NCORES          # batches per core
G = 50                    # ground truths per image
C = 16                    # num classes
CH = 5 + C                # channels (x, y, w, h, obj, cls...)
STRIDES = (8, 16, 32)
WS = (128, 64, 32)
HWS = (128 * 128, 64 * 64, 32 * 32)
A = sum(HWS)              # 21504 anchors
BASES = (0, HWS[0], HWS[0] + HWS[1])
NB = BL * 3               # (batch, scale) blocks; partition p = b*3 + s
BLK = 64                  # columns reserved per block (50 real + 14 pad)
NCOL = NB * BLK           # 768
NW = NCOL // 128          # 6 winner-major blocks
EPS = 1e-8
INPUT_W = 1024.0


def _build_program():
    nc = bacc.Bacc("TRN2", target_bir_lowering=False)

    cl_d = nc.dram_tensor("cl", [BL, A, CH], F32, kind="ExternalInput")
    obj_d = [
        nc.dram_tensor(f"obj{s}", [BL * HWS[s]], F32, kind="ExternalInput")
        for s in range(3)
    ]
    labels_d = nc.dram_tensor("labels", [BL, G, 5], F32, kind="ExternalInput")
    cst_d = nc.dram_tensor("cst", [NB, 2], F32, kind="ExternalInput")
    boff_d = nc.dram_tensor("boffr", [1, NCOL], F32, kind="ExternalInput")
    wc4_d = nc.dram_tensor("wc4", [128, NW * 4], F32, kind="ExternalInput")
    part_d = nc.dram_tensor("partials", [8], F32, kind="ExternalOutput")

    with tile.TileContext(nc) as tc:
        _body(tc, cl_d, obj_d, labels_d, cst_d, boff_d, wc4_d, part_d)
    if not nc.is_finalized():
        nc.finalize()
    return nc


def _body(tc, cl_d, obj_d, labels_d, cst_d, boff_d, wc4_d, part_d):
    nc = tc.nc
    from contextlib import ExitStack

    ctx = ExitStack()
    sg = ctx.enter_context(tc.tile_pool(name="sg", bufs=1))
    psum = ctx.enter_context(tc.tile_pool(name="psum", bufs=1, space="PSUM"))

    # ---------------- constants ----------------
    cst = sg.tile([NB, 2], F32)          # [:,0] 1/stride, [:,1] W
    nc.sync.dma_start(cst[:], cst_d[:])
    boffr = sg.tile([1, NCOL], F32)      # b*A + base_s per column
    nc.sync.dma_start(boffr[:], boff_d[:])
    WC4 = sg.tile([128, NW, 4], F32)     # vec weights winner-major
    nc.sync.dma_start(WC4[:], wc4_d[:].rearrange("p (i c) -> p i c", c=4))

    ident = sg.tile([NB, NB], F32)
    make_identity(nc, ident[:])
    ones128 = sg.tile([128, 1], F32)
    nc.vector.memset(ones128[:], 1.0)
    onesr50 = sg.tile([1, G], F32)
    nc.vector.memset(onesr50[:], 1.0)
    ones50 = sg.tile([G, 1], F32)
    nc.vector.memset(ones50[:], 1.0)
    epsb = sg.tile([128, 1], F32)
    nc.vector.memset(epsb[:], EPS)

    cvals_i = sg.tile([128, C], I32)     # 0..15 along free dim
    nc.gpsimd.iota(cvals_i[:], pattern=[[1, C]], base=0, channel_multiplier=0)
    cvals = sg.tile([128, C], F32)
    nc.vector.tensor_copy(cvals[:], cvals_i[:])

    # lowm[g', g] = 1.0 if g' < g else 0.0   [G, BLK]
    lowm_i = sg.tile([G, BLK], I32)
    nc.gpsimd.iota(lowm_i[:], pattern=[[-1, BLK]], base=0, channel_multiplier=1)
    lowm_f = sg.tile([G, BLK], F32)
    nc.vector.tensor_copy(lowm_f[:], lowm_i[:])
    lowm = sg.tile([G, BLK], F32)
    nc.vector.tensor_single_scalar(lowm[:], lowm_f[:], 0.0, OP.is_lt)

    # ---------------- labels ----------------
    lab34 = sg.tile([2, BL, G], F32)     # w, h targets
    nc.sync.dma_start(lab34[:], labels_d[:, :, 2:4].rearrange("b g c -> c b g"))
    labC = sg.tile([1, BL, G], F32)      # class ids
    nc.sync.dma_start(labC[:], labels_d[:, :, 4:5].rearrange("b g c -> c b g"))
    XY = sg.tile([NB, G, 2], F32)        # p = b*3+s
    for s in range(3):
        out_ap = XY[:].rearrange("(b s) g c -> b s g c", s=3)[:, s, :, :]
        nc.sync.dma_start(out_ap, labels_d[:, :, 0:2])

    # ---------------- stage A: cells & validity  [NB, G, 2] ----------------
    U = sg.tile([NB, G, 2], F32)
    nc.vector.tensor_scalar(U[:], XY[:], cst[:, 0:1], None, op0=OP.mult)
    T1 = sg.tile([NB, G, 2], F32)
    nc.vector.tensor_single_scalar(T1[:], U[:], 0.5, OP.subtract)
    AXi = sg.tile([NB, G, 2], I32)
    nc.vector.tensor_copy(AXi[:], T1[:])          # ~floor (any rounding, +-1 ok)
    AXf = sg.tile([NB, G, 2], F32)
    nc.vector.tensor_copy(AXf[:], AXi[:])
    # exact fix: ax = ax0 + (u >= ax0+1) - (u < ax0)
    T2 = sg.tile([NB, G, 2], F32)
    nc.vector.tensor_single_scalar(T1[:], AXf[:], 1.0, OP.add)
    nc.vector.tensor_tensor(T2[:], U[:], T1[:], OP.is_ge)
    nc.vector.tensor_tensor(AXf[:], AXf[:], T2[:], OP.add)
    nc.vector.tensor_tensor(T2[:], U[:], AXf[:], OP.is_lt)
    nc.vector.tensor_tensor(AXf[:], AXf[:], T2[:], OP.subtract)
    # in-bounds (strict): ax < u < ax+1
    nc.vector.tensor_single_scalar(T1[:], AXf[:], 1.0, OP.add)
    nc.vector.tensor_tensor(T2[:], U[:], AXf[:], OP.is_gt)
    INB = sg.tile([NB, G, 2], F32)
    nc.vector.tensor_tensor(INB[:], U[:], T1[:], OP.is_lt)
    nc.vector.tensor_tensor(INB[:], INB[:], T2[:], OP.mult)

    valid = sg.tile([NB, G], F32)
    nc.vector.tensor_tensor(valid[:], INB[:, :, 0], INB[:, :, 1], OP.mult)
    UAX = sg.tile([NB, G, 2], F32)       # u - ax (decode offsets)
    nc.vector.tensor_tensor(UAX[:], U[:], AXf[:], OP.subtract)
    cell = sg.tile([NB, G], F32)         # ay*W + ax
    nc.vector.tensor_scalar(cell[:], AXf[:, :, 1], cst[:, 1:2], None, op0=OP.mult)
    nc.vector.tensor_tensor(cell[:], cell[:], AXf[:, :, 0], OP.add)

    # ---------------- zero-padded [12, 64] staging tiles ----------------
    def p64(src_ap, name):
        t = sg.tile([NB, BLK], F32, name=name)
        nc.vector.memset(t[:], 0.0)
        nc.vector.tensor_copy(t[:, 0:G], src_ap)
        return t

    valid64 = p64(valid[:], "valid64")
    cell64 = p64(cell[:], "cell64")
    u064 = p64(UAX[:, :, 0], "u064")
    u164 = p64(UAX[:, :, 1], "u164")
    tv64 = sg.tile([2, BL, 3, BLK], F32)
    nc.vector.memset(tv64[:], 0.0)
    nc.vector.tensor_copy(
        tv64[:, :, :, 0:G], lab34[:].unsqueeze(2).broadcast_to([2, BL, 3, G])
    )
    cls64 = sg.tile([1, BL, 3, BLK], F32)
    nc.vector.memset(cls64[:], 0.0)
    nc.vector.tensor_copy(
        cls64[:, :, :, 0:G], labC[:].unsqueeze(2).broadcast_to([1, BL, 3, G])
    )

    # meta rows staged to DRAM: 0 win | 1 u0 | 2 u1 | 3 t2 | 4 t3 | 5 cls
    meta_d = nc.dram_tensor("meta_scratch", [6, NCOL], F32)
    nc.sync.dma_start(meta_d[1:2], u064[:])
    nc.sync.dma_start(meta_d[2:3], u164[:])
    nc.sync.dma_start(meta_d[3:5], tv64[:].rearrange("p b s k -> p (b s k)"))
    nc.sync.dma_start(meta_d[5:6], cls64[:].rearrange("p b s k -> p (b s k)"))

    # row layout [1, NCOL] for the winner resolution
    VALIDR = sg.tile([1, NCOL], F32)
    nc.sync.dma_start(VALIDR[:], valid64[:])
    CELLR = sg.tile([1, NCOL], F32)
    nc.sync.dma_start(CELLR[:], cell64[:])

    # ---------------- winner resolution ----------------
    cvTp = psum.tile([G, NB], F32)
    nc.tensor.transpose(cvTp[:], cell[:], ident[:])
    vvTp = psum.tile([G, NB], F32)
    nc.tensor.transpose(vvTp[:], valid[:], ident[:])
    cvT = sg.tile([G, 2, NB], F32)
    nc.vector.tensor_copy(cvT[:, 0, :], cvTp[:])
    nc.vector.tensor_copy(cvT[:, 1, :], vvTp[:])

    crow_ps = psum.tile([G, NCOL], F32, tag="wide", bufs=2)
    for lo in range(0, NCOL, 512):
        hi = min(lo + 512, NCOL)
        nc.tensor.matmul(crow_ps[:, lo:hi], lhsT=onesr50[:], rhs=CELLR[0:1, lo:hi],
                         start=True, stop=True)

    # E2[g', (bs,g)] = valid[bs,g'] * (cell[bs,g'] == cell[bs,g]) * (g' < g)
    E2 = sg.tile([G, NB, BLK], F32)
    cellT_b = cvT[:, 0, :].unsqueeze(2).broadcast_to([G, NB, BLK])
    validT_b = cvT[:, 1, :].unsqueeze(2).broadcast_to([G, NB, BLK])
    cell_row = crow_ps[:].rearrange("g (n k) -> g n k", k=BLK)
    lowm_b = lowm[:].unsqueeze(1).broadcast_to([G, NB, BLK])
    nc.vector.tensor_tensor(E2[:], cellT_b, cell_row, OP.is_equal)
    nc.vector.tensor_tensor(E2[:], E2[:], validT_b, OP.mult)
    nc.vector.tensor_tensor(E2[:], E2[:], lowm_b, OP.mult)

    E2f = E2[:].rearrange("g n k -> g (n k)")
    cpsum = psum.tile([1, NCOL], F32, tag="wide", bufs=2)
    for lo in range(0, NCOL, 512):
        hi = min(lo + 512, NCOL)
        nc.tensor.matmul(cpsum[:, lo:hi], lhsT=ones50[:], rhs=E2f[:, lo:hi],
                         start=True, stop=True)

    WINR = sg.tile([1, NCOL], F32)       # win = valid & (conflict == 0)
    nc.vector.tensor_single_scalar(WINR[:], cpsum[:], 0.0, OP.is_equal)
    nc.vector.tensor_tensor(WINR[:], WINR[:], VALIDR[0:1], OP.mult)
    nc.sync.dma_start(meta_d[0:1], WINR[:])

    # gather offsets: win * (b*A + base_s + cell); losers/pads -> row 0
    IDXF = sg.tile([1, NCOL], F32)
    nc.vector.tensor_tensor(IDXF[:], CELLR[0:1], boffr[0:1], OP.add)
    nc.vector.tensor_tensor(IDXF[:], IDXF[:], WINR[:], OP.mult)
    idx32 = sg.tile([1, NCOL], I32)
    nc.vector.tensor_copy(idx32[:], IDXF[:])
    idx_d = nc.dram_tensor("idx_scratch", [NCOL], I32)
    nc.sync.dma_start(idx_d[:], idx32[:])

    # winner-major: j = i*128 + p
    offw = sg.tile([128, NW], I32)
    nc.sync.dma_start(offw[:], idx_d[:].rearrange("(i p) -> p i", p=128))
    # MW[p, r, i]: meta row r for winner j = i*128 + p
    MW = sg.tile([128, 6, NW], F32)
    for r in range(6):
        nc.sync.dma_start(
            MW[:, r, :], meta_d[r : r + 1].rearrange("r (i p) -> (r p) i", p=128)
        )

    # ---------------- gather winner rows from DRAM ----------------
    G6 = sg.tile([128, NW, CH], F32)
    for i in range(NW):
        nc.gpsimd.indirect_dma_start(
            out=G6[:, i, :],
            out_offset=None,
            in_=cl_d[:],
            in_offset=bass.IndirectOffsetOnAxis(ap=offw[:, i : i + 1], axis=1),
        )

    # ---------------- loss, winner-major ----------------
    SCAL = sg.tile([128, 8], F32)
    nc.vector.memset(SCAL[:], 0.0)
    winw = MW[:, 0, :]                       # [128, NW]

    # vec: D_c = meta_c - G_c, weighted |.| sums
    D4 = sg.tile([128, NW, 4], F32)
    for c in range(4):
        nc.vector.tensor_tensor(D4[:, :, c], MW[:, 1 + c, :], G6[:, :, c],
                                OP.subtract)
    WM = sg.tile([128, NW, 4], F32)
    nc.vector.tensor_tensor(
        WM[:], WC4[:], winw.unsqueeze(2).broadcast_to([128, NW, 4]), OP.mult
    )
    nc.vector.tensor_tensor(D4[:], D4[:], WM[:], OP.mult)
    nc.vector.tensor_reduce(
        SCAL[:, 0:4], D4[:].rearrange("p i c -> p c i"), axis=AX.X, op=OP.add,
        apply_absolute_value=True,
    )

    # obj foreground: sum win * (1000*ln(obj+eps) - ln(1-obj+eps))
    OT = sg.tile([128, NW], F32)
    nc.vector.tensor_scalar(OT[:], G6[:, :, 4], -1.0, 1.0, op0=OP.mult, op1=OP.add)
    LPO = sg.tile([128, NW], F32)
    nc.scalar.activation(LPO[:], G6[:, :, 4], AF.Ln, bias=epsb[:, 0:1], scale=1.0)
    nc.scalar.activation(OT[:], OT[:], AF.Ln, bias=epsb[:, 0:1], scale=1.0)
    nc.vector.scalar_tensor_tensor(OT[:], LPO[:], 1000.0, OT[:],
                                   op0=OP.mult, op1=OP.subtract)
    nc.vector.tensor_tensor(OT[:], OT[:], winw, OP.mult)
    nc.vector.tensor_reduce(SCAL[:, 4:5], OT[:], axis=AX.X, op=OP.add)

    # cls: sum win * sum_c [lq + ct*(lp - lq)]   (clamped logs)
    TC = sg.tile([128, NW, C], F32)
    nc.vector.tensor_scalar(TC[:], G6[:, :, 5:CH], -1.0, 1.0, op0=OP.mult, op1=OP.add)
    LPC = sg.tile([128, NW, C], F32)
    nc.scalar.activation(LPC[:], G6[:, :, 5:CH], AF.Ln, bias=0.0, scale=1.0)
    nc.scalar.activation(TC[:], TC[:], AF.Ln, bias=0.0, scale=1.0)
    nc.vector.tensor_single_scalar(LPC[:], LPC[:], -100.0, OP.max)
    nc.vector.tensor_single_scalar(TC[:], TC[:], -100.0, OP.max)
    CT = sg.tile([128, NW, C], F32)
    nc.vector.tensor_tensor(
        CT[:],
        MW[:, 5, :].unsqueeze(2).broadcast_to([128, NW, C]),
        cvals[:].unsqueeze(1).broadcast_to([128, NW, C]),
        OP.is_equal,
    )
    nc.vector.tensor_tensor(LPC[:], LPC[:], TC[:], OP.subtract)      # d
    nc.vector.tensor_tensor(CT[:], CT[:], LPC[:], OP.mult)           # ct*d
    nc.vector.tensor_tensor(CT[:], CT[:], TC[:], OP.add)             # e
    ECOL = sg.tile([128, NW], F32)
    nc.vector.tensor_reduce(ECOL[:], CT[:], axis=AX.X, op=OP.add)
    nc.vector.tensor_tensor(ECOL[:], ECOL[:], winw, OP.mult)
    nc.vector.tensor_reduce(SCAL[:, 5:6], ECOL[:], axis=AX.X, op=OP.add)

    # ---------------- dense obj background ----------------
    OBJ = sg.tile([128, (BL * A) // 128], F32)   # 672 cols
    col0 = 0
    for s in range(3):
        k = BL * HWS[s] // 128
        nc.sync.dma_start(
            OBJ[:, col0 : col0 + k], obj_d[s][:].rearrange("(p j) -> p j", p=128)
        )
        col0 += k
    nc.vector.tensor_scalar(OBJ[:], OBJ[:], -1.0, 1.0, op0=OP.mult, op1=OP.add)
    dacc = sg.tile([128, 1], F32)
    nc.scalar.activation(OBJ[:], OBJ[:], AF.Ln, bias=epsb[:, 0:1], scale=1.0,
                         accum_out=dacc[:])
    nc.vector.tensor_copy(SCAL[:, 6:7], dacc[:])

    # ---------------- partition reduce + output ----------------
    spsum = psum.tile([8, 1], F32)
    nc.tensor.matmul(spsum[:], lhsT=SCAL[:], rhs=ones128[:], start=True, stop=True)
    sout = sg.tile([8, 1], F32)
    nc.vector.tensor_copy(sout[:], spsum[:])
    nc.sync.dma_start(part_d[:], sout[:])

    ctx.close()


_PROG = None
TRACE = False
LAST_RESULTS = None


def _get_program():
    global _PROG
    if _PROG is None:
        _PROG = _build_program()
    return _PROG


def _host_constants():
    cst = np.zeros((NB, 2), np.float32)
    for b in range(BL):
        for s in range(3):
            cst[b * 3 + s, 0] = 1.0 / STRIDES[s]
            cst[b * 3 + s, 1] = WS[s]
    boff = np.zeros((1, NCOL), np.float32)
    sw = np.zeros((NCOL,), np.float32)
    for b in range(BL):
        for s in range(3):
            blk = b * 3 + s
            boff[0, blk * BLK : (blk + 1) * BLK] = b * A + BASES[s]
            sw[blk * BLK : (blk + 1) * BLK] = STRIDES[s] / INPUT_W
    # winner-major vec weights: j = i*128 + p -> [p, i, c]
    wc4 = np.zeros((128, NW, 4), np.float32)
    for j in range(NCOL):
        p, i = j % 128, j // 128
        wc4[p, i, 0] = wc4[p, i, 1] = sw[j]
        wc4[p, i, 2] = wc4[p, i, 3] = 10.0
    return cst, boff, wc4.reshape(128, NW * 4)


def kernel(input0, input1, input2, labels):
    nc = _get_program()
    cst, boff, wc4 = _host_constants()
    # channels-last rows, all scales concatenated along the anchor dim
    ins = (input0, input1, input2)
    cl = np.concatenate(
        [
            np.asarray(t, np.float32)
            .transpose(0, 2, 3, 1)
            .reshape(B, -1, CH)
            for t in ins
        ],
        axis=1,
    )
    objs = [np.ascontiguousarray(np.asarray(t, np.float32)[:, 4]).reshape(B, -1)
            for t in ins]
    in_maps = []
    for c in range(NCORES):
        sl = slice(c * BL, (c + 1) * BL)
        m = {
            "cl": np.ascontiguousarray(cl[sl]),
            "labels": np.ascontiguousarray(np.asarray(labels, np.float32)[sl]),
            "cst": cst,
            "boffr": boff,
            "wc4": wc4,
        }
        for s in range(3):
            m[f"obj{s}"] = np.ascontiguousarray(objs[s][sl]).reshape(-1)
        in_maps.append(m)
    global LAST_RESULTS
    res = run_bass_kernel_spmd(
        nc, in_maps, core_ids=list(range(NCORES)), trace=TRACE
    )
    LAST_RESULTS = res
    P = np.stack([r["partials"] for r in res.results]).astype(np.float64)
    a_sum = P[:, 0:4].sum(axis=0)
    loss_obj = -(P[:, 4].sum() + P[:, 6].sum())
    loss_cls = -500.0 * P[:, 5].sum()
    loss_vec = a_sum.sum() * 10000.0
    loss = np.float32(loss_obj + loss_vec + loss_cls)
    return (np.array(loss, np.float32), a_sum.astype(np.float32))


if __name__ == "__main__":
    _get_program()
    print("program built OK")


# revision 26
# speedup vs baseline: 2.4059x; 1.0628x over previous
"""Trainium2 Bass kernel for nn_DPSV_Loss (YOLOX-style detection loss).

Strategy
--------
Data parallel over batch: B=32 images sharded 4-per-core across 8 NeuronCores.

Key algorithmic insight: the center-radius assignment uses r = 0.5*stride, so
each ground-truth box matches EXACTLY ONE grid cell per FPN scale (the cell
containing its center), or none when the center sits exactly on a cell
boundary (strict inequalities).  First-match (argmax over gts) semantics
reduce to "earliest valid gt wins each cell", resolvable on a tiny conflict
matrix over gt pairs.  The dense [B, G, A] candidate tensor of the reference
never materializes, and of the 21*A inputs per image only the objectness
channel is ever needed densely — everything else is consumed at the <=150
foreground anchors per image.

Per core the device program:
  1. computes cell/validity per (batch, scale, gt)          [12, 50] tiles
  2. resolves first-match winners via an equality/conflict matrix
     column-summed on the TensorEngine                      [50, 768]
  3. gathers the 21 channels at winner anchors straight from DRAM
     (channels-last staging => one 84B contiguous row per winner)
  4. evaluates vec/obj/cls loss terms winner-major          [128, 6, *]
  5. densely reduces only log(1-obj+eps) over all anchors   [128, 672]
  6. emits 7 partial sums via one [128, 8] x ones matmul; host combines
     the 8 cores.

The host-side kernel() only re-lays-out inputs (transpose to channels-last +
a contiguous copy of the obj channel) and shards the batch; all arithmetic
happens on device.
"""

import sys

_TRN_REPO = "/opt/trn_rl_repo"
if _TRN_REPO not in sys.path:
    sys.path.insert(0, _TRN_REPO)

import numpy as np

import concourse.bass as bass
import concourse.bacc as bacc
import concourse.tile as tile
from concourse import mybir
from concourse.bass_utils import run_bass_kernel_spmd
from concourse.masks import make_identity

F32 = mybir.dt.float32
I32 = mybir.dt.int32
OP = mybir.AluOpType
AF = mybir.ActivationFunctionType
AX = mybir.AxisListType

NCORES = 8
B = 32
BL = B // NCORES          # batches per core
G = 50                    # ground truths per image
C = 16                    # num classes
CH = 5 + C                # channels (x, y, w, h, obj, cls...)
STRIDES = (8, 16, 32)
WS = (128, 64, 32)
HWS = (128 * 128, 64 * 64, 32 * 32)
A = sum(HWS)              # 21504 anchors
BASES = (0, HWS[0], HWS[0] + HWS[1])
NB = BL * 3               # (batch, scale) blocks; partition p = b*3 + s
BLK = 64                  # columns reserved per block (50 real + 14 pad)
NCOL = NB * BLK           # 768
NW = NCOL // 128          # 6 winner-major blocks
EPS = 1e-8
INPUT_W = 1024.0


def _build_program():
    nc = bacc.Bacc("TRN2", target_bir_lowering=False)

    cl_d = nc.dram_tensor("cl", [BL, A, CH], F32, kind="ExternalInput")
    obj_d = [
        nc.dram_tensor(f"obj{s}", [BL * HWS[s]], F32, kind="ExternalInput")
        for s in range(3)
    ]
    labels_d = nc.dram_tensor("labels", [BL, G, 5], F32, kind="ExternalInput")
    cst_d = nc.dram_tensor("cst", [NB, 3], F32, kind="ExternalInput")
    wc4_d = nc.dram_tensor("wc4", [128, NW * 4], F32, kind="ExternalInput")
    part_d = nc.dram_tensor("partials", [8], F32, kind="ExternalOutput")

    with tile.TileContext(nc) as tc:
        _body(tc, cl_d, obj_d, labels_d, cst_d, wc4_d, part_d)
    if not nc.is_finalized():
        nc.finalize()
    return nc


def _body(tc, cl_d, obj_d, labels_d, cst_d, wc4_d, part_d):
    nc = tc.nc
    from contextlib import ExitStack

    ctx = ExitStack()
    sg = ctx.enter_context(tc.tile_pool(name="sg", bufs=1))
    psum = ctx.enter_context(tc.tile_pool(name="psum", bufs=1, space="PSUM"))

    # ---------------- constants ----------------
    cst = sg.tile([NB, 3], F32)          # 1/stride | W | b*A + base_s
    nc.scalar.dma_start(cst[:], cst_d[:])
    WC4 = sg.tile([128, NW, 4], F32)     # vec weights winner-major
    nc.scalar.dma_start(WC4[:], wc4_d[:].rearrange("p (i c) -> p i c", c=4))

    ident = sg.tile([NB, NB], F32)
    make_identity(nc, ident[:])
    ones128 = sg.tile([128, 1], F32)
    nc.vector.memset(ones128[:], 1.0)
    onesr50 = sg.tile([1, G], F32)
    nc.vector.memset(onesr50[:], 1.0)
    ones50 = sg.tile([G, 1], F32)
    nc.vector.memset(ones50[:], 1.0)
    epsb = sg.tile([128, 1], F32)
    nc.vector.memset(epsb[:], EPS)

    cvals_i = sg.tile([128, C], I32)     # 0..15 along free dim
    nc.gpsimd.iota(cvals_i[:], pattern=[[1, C]], base=0, channel_multiplier=0)
    cvals = sg.tile([128, C], F32)
    nc.vector.tensor_copy(cvals[:], cvals_i[:])

    # lowm[g', g] = 1.0 if g' < g else 0.0   [G, BLK]
    lowm_i = sg.tile([G, BLK], I32)
    nc.gpsimd.iota(lowm_i[:], pattern=[[-1, BLK]], base=0, channel_multiplier=1)
    lowm_f = sg.tile([G, BLK], F32)
    nc.vector.tensor_copy(lowm_f[:], lowm_i[:])
    lowm = sg.tile([G, BLK], F32)
    nc.vector.tensor_single_scalar(lowm[:], lowm_f[:], 0.0, OP.is_lt)

    # ---------------- labels ----------------
    lab34 = sg.tile([2, BL, G], F32)     # w, h targets
    nc.scalar.dma_start(lab34[:], labels_d[:, :, 2:4].rearrange("b g c -> c b g"))
    labC = sg.tile([1, BL, G], F32)      # class ids
    nc.gpsimd.dma_start(labC[:], labels_d[:, :, 4:5].rearrange("b g c -> c b g"))
    XY = sg.tile([NB, G, 2], F32)        # p = b*3+s
    for s in range(3):
        out_ap = XY[:].rearrange("(b s) g c -> b s g c", s=3)[:, s, :, :]
        nc.sync.dma_start(out_ap, labels_d[:, :, 0:2])


    # ---------------- stage A: cells & validity  [NB, G, 2] ----------------
    U = sg.tile([NB, G, 2], F32)
    nc.vector.tensor_scalar(U[:], XY[:], cst[:, 0:1], None, op0=OP.mult)
    T1 = sg.tile([NB, G, 2], F32)
    nc.vector.tensor_single_scalar(T1[:], U[:], 0.5, OP.subtract)
    AXi = sg.tile([NB, G, 2], I32)
    nc.vector.tensor_copy(AXi[:], T1[:])          # ~floor (any rounding, +-1 ok)
    AXf = sg.tile([NB, G, 2], F32)
    nc.vector.tensor_copy(AXf[:], AXi[:])
    # exact fix: ax = ax0 + (u >= ax0+1) - (u < ax0)
    T2 = sg.tile([NB, G, 2], F32)
    nc.vector.tensor_single_scalar(T1[:], AXf[:], 1.0, OP.add)
    nc.vector.tensor_tensor(T2[:], U[:], T1[:], OP.is_ge)
    nc.vector.tensor_tensor(AXf[:], AXf[:], T2[:], OP.add)
    nc.vector.tensor_tensor(T2[:], U[:], AXf[:], OP.is_lt)
    nc.vector.tensor_tensor(AXf[:], AXf[:], T2[:], OP.subtract)
    # in-bounds (strict): ax < u < ax+1
    nc.vector.tensor_single_scalar(T1[:], AXf[:], 1.0, OP.add)
    nc.vector.tensor_tensor(T2[:], U[:], AXf[:], OP.is_gt)
    INB = sg.tile([NB, G, 2], F32)
    nc.vector.tensor_tensor(INB[:], U[:], T1[:], OP.is_lt)
    nc.vector.tensor_tensor(INB[:], INB[:], T2[:], OP.mult)

    valid = sg.tile([NB, G], F32)
    nc.vector.tensor_tensor(valid[:], INB[:, :, 0], INB[:, :, 1], OP.mult)
    UAX = sg.tile([NB, G, 2], F32)       # u - ax (decode offsets)
    nc.vector.tensor_tensor(UAX[:], U[:], AXf[:], OP.subtract)
    cell = sg.tile([NB, G], F32)         # global: b*A + base_s + ay*W + ax
    nc.vector.tensor_scalar(cell[:], AXf[:, :, 1], cst[:, 1:2], cst[:, 2:3],
                            op0=OP.mult, op1=OP.add)
    nc.vector.tensor_tensor(cell[:], cell[:], AXf[:, :, 0], OP.add)

    # ---------------- zero-padded [12, 64] staging tiles ----------------
    def p64(src_ap, name):
        t = sg.tile([NB, BLK], F32, name=name)
        nc.vector.memset(t[:], 0.0)
        nc.vector.tensor_copy(t[:, 0:G], src_ap)
        return t

    valid64 = p64(valid[:], "valid64")
    cell64 = p64(cell[:], "cell64")
    u064 = p64(UAX[:, :, 0], "u064")
    u164 = p64(UAX[:, :, 1], "u164")
    tv64 = sg.tile([2, BL, 3, BLK], F32)
    nc.vector.memset(tv64[:], 0.0)
    nc.vector.tensor_copy(
        tv64[:, :, :, 0:G], lab34[:].unsqueeze(2).broadcast_to([2, BL, 3, G])
    )
    cls64 = sg.tile([1, BL, 3, BLK], F32)
    nc.vector.memset(cls64[:], 0.0)
    nc.vector.tensor_copy(
        cls64[:, :, :, 0:G], labC[:].unsqueeze(2).broadcast_to([1, BL, 3, G])
    )

    # meta rows staged to DRAM: 0 win | 1 u0 | 2 u1 | 3 t2 | 4 t3 | 5 cls
    meta_d = nc.dram_tensor("meta_scratch", [6, NCOL], F32)
    nc.gpsimd.dma_start(meta_d[1:2], u064[:])
    nc.scalar.dma_start(meta_d[2:3], u164[:])
    nc.scalar.dma_start(meta_d[3:5], tv64[:].rearrange("p b s k -> p (b s k)"))
    nc.gpsimd.dma_start(meta_d[5:6], cls64[:].rearrange("p b s k -> p (b s k)"))

    # row layout [1, NCOL] for the winner resolution
    VALIDR = sg.tile([1, NCOL], F32)
    nc.sync.dma_start(VALIDR[:], valid64[:])
    CELLR = sg.tile([1, NCOL], F32)
    nc.gpsimd.dma_start(CELLR[:], cell64[:])

    # ---------------- winner resolution ----------------
    cvTp = psum.tile([G, NB], F32)
    nc.tensor.transpose(cvTp[:], cell[:], ident[:])
    vvTp = psum.tile([G, NB], F32)
    nc.tensor.transpose(vvTp[:], valid[:], ident[:])
    cvT = sg.tile([G, 2, NB], F32)
    nc.vector.tensor_copy(cvT[:, 0, :], cvTp[:])
    nc.vector.tensor_copy(cvT[:, 1, :], vvTp[:])

    crow_ps = psum.tile([G, NCOL], F32, tag="wide", bufs=2)
    for lo in range(0, NCOL, 512):
        hi = min(lo + 512, NCOL)
        nc.tensor.matmul(crow_ps[:, lo:hi], lhsT=onesr50[:], rhs=CELLR[0:1, lo:hi],
                         start=True, stop=True)

    # E2[g', (bs,g)] = valid[bs,g'] * (cell[bs,g'] == cell[bs,g]) * (g' < g)
    E2 = sg.tile([G, NB, BLK], F32)
    cellT_b = cvT[:, 0, :].unsqueeze(2).broadcast_to([G, NB, BLK])
    validT_b = cvT[:, 1, :].unsqueeze(2).broadcast_to([G, NB, BLK])
    cell_row = crow_ps[:].rearrange("g (n k) -> g n k", k=BLK)
    lowm_b = lowm[:].unsqueeze(1).broadcast_to([G, NB, BLK])
    nc.vector.tensor_tensor(E2[:], cellT_b, cell_row, OP.is_equal)
    nc.vector.tensor_tensor(E2[:], E2[:], validT_b, OP.mult)
    nc.vector.tensor_tensor(E2[:], E2[:], lowm_b, OP.mult)

    E2f = E2[:].rearrange("g n k -> g (n k)")
    cpsum = psum.tile([1, NCOL], F32, tag="wide", bufs=2)
    for lo in range(0, NCOL, 512):
        hi = min(lo + 512, NCOL)
        nc.tensor.matmul(cpsum[:, lo:hi], lhsT=ones50[:], rhs=E2f[:, lo:hi],
                         start=True, stop=True)

    WINR = sg.tile([1, NCOL], F32)       # win = valid & (conflict == 0)
    nc.vector.tensor_single_scalar(WINR[:], cpsum[:], 0.0, OP.is_equal)
    nc.vector.tensor_tensor(WINR[:], WINR[:], VALIDR[0:1], OP.mult)
    nc.sync.dma_start(meta_d[0:1], WINR[:])

    # gather offsets: win * (b*A + base_s + cell); losers/pads -> row 0
    IDXF = sg.tile([1, NCOL], F32)
    nc.vector.tensor_tensor(IDXF[:], CELLR[0:1], WINR[:], OP.mult)
    idx32 = sg.tile([1, NCOL], I32)
    nc.vector.tensor_copy(idx32[:], IDXF[:])
    idx_d = nc.dram_tensor("idx_scratch", [NCOL], I32)
    nc.sync.dma_start(idx_d[:], idx32[:])

    # winner-major: j = i*128 + p
    offw = sg.tile([128, NW], I32)
    nc.sync.dma_start(offw[:], idx_d[:].rearrange("(i p) -> p i", p=128))
    # MW[p, r, i]: meta row r for winner j = i*128 + p
    MW = sg.tile([128, 6, NW], F32)
    _eng = [nc.sync, nc.scalar, nc.gpsimd, nc.sync, nc.scalar, nc.gpsimd]
    for r in range(6):
        _eng[r].dma_start(
            MW[:, r, :], meta_d[r : r + 1].rearrange("r (i p) -> (r p) i", p=128)
        )

    # ---------------- gather winner rows from DRAM ----------------
    G6 = sg.tile([128, NW, CH], F32)
    for i in range(NW):
        nc.gpsimd.indirect_dma_start(
            out=G6[:, i, :],
            out_offset=None,
            in_=cl_d[:],
            in_offset=bass.IndirectOffsetOnAxis(ap=offw[:, i : i + 1], axis=1),
        )

    # ---------------- loss, winner-major ----------------
    SCAL = sg.tile([128, 8], F32)
    nc.vector.memset(SCAL[:], 0.0)
    winw = MW[:, 0, :]                       # [128, NW]

    # vec: D_c = meta_c - G_c, weighted |.| sums
    D4 = sg.tile([128, NW, 4], F32)
    for c in range(4):
        nc.vector.tensor_tensor(D4[:, :, c], MW[:, 1 + c, :], G6[:, :, c],
                                OP.subtract)
    WM = sg.tile([128, NW, 4], F32)
    nc.vector.tensor_tensor(
        WM[:], WC4[:], winw.unsqueeze(2).broadcast_to([128, NW, 4]), OP.mult
    )
    nc.vector.tensor_tensor(D4[:], D4[:], WM[:], OP.mult)
    nc.vector.tensor_reduce(
        SCAL[:, 0:4], D4[:].rearrange("p i c -> p c i"), axis=AX.X, op=OP.add,
        apply_absolute_value=True,
    )

    # obj foreground: sum win * (1000*ln(obj+eps) - ln(1-obj+eps))
    OT = sg.tile([128, NW], F32)
    nc.vector.tensor_scalar(OT[:], G6[:, :, 4], -1.0, 1.0, op0=OP.mult, op1=OP.add)
    LPO = sg.tile([128, NW], F32)
    nc.scalar.activation(LPO[:], G6[:, :, 4], AF.Ln, bias=epsb[:, 0:1], scale=1.0)
    nc.scalar.activation(OT[:], OT[:], AF.Ln, bias=epsb[:, 0:1], scale=1.0)
    nc.vector.scalar_tensor_tensor(OT[:], LPO[:], 1000.0, OT[:],
                                   op0=OP.mult, op1=OP.subtract)
    nc.vector.tensor_tensor(OT[:], OT[:], winw, OP.mult)
    nc.vector.tensor_reduce(SCAL[:, 4:5], OT[:], axis=AX.X, op=OP.add)

    # cls: sum win * sum_c [lq + ct*(lp - lq)]   (clamped logs)
    TC = sg.tile([128, NW, C], F32)
    nc.vector.tensor_scalar(TC[:], G6[:, :, 5:CH], -1.0, 1.0, op0=OP.mult, op1=OP.add)
    LPC = sg.tile([128, NW, C], F32)
    nc.scalar.activation(LPC[:], G6[:, :, 5:CH], AF.Ln, bias=0.0, scale=1.0)
    nc.scalar.activation(TC[:], TC[:], AF.Ln, bias=0.0, scale=1.0)
    nc.vector.tensor_single_scalar(LPC[:], LPC[:], -100.0, OP.max)
    nc.vector.tensor_single_scalar(TC[:], TC[:], -100.0, OP.max)
    CT = sg.tile([128, NW, C], F32)
    nc.vector.tensor_tensor(
        CT[:],
        MW[:, 5, :].unsqueeze(2).broadcast_to([128, NW, C]),
        cvals[:].unsqueeze(1).broadcast_to([128, NW, C]),
        OP.is_equal,
    )
    nc.vector.tensor_tensor(LPC[:], LPC[:], TC[:], OP.subtract)      # d
    nc.vector.tensor_tensor(CT[:], CT[:], LPC[:], OP.mult)           # ct*d
    nc.vector.tensor_tensor(CT[:], CT[:], TC[:], OP.add)             # e
    ECOL = sg.tile([128, NW], F32)
    nc.vector.tensor_reduce(ECOL[:], CT[:], axis=AX.X, op=OP.add)
    nc.vector.tensor_tensor(ECOL[:], ECOL[:], winw, OP.mult)
    nc.vector.tensor_reduce(SCAL[:, 5:6], ECOL[:], axis=AX.X, op=OP.add)

    # ---------------- dense obj background ----------------
    OBJ = sg.tile([128, (BL * A) // 128], F32)   # 672 cols
    col0 = 0
    for s in range(3):
        k = BL * HWS[s] // 128
        nc.scalar.dma_start(
            OBJ[:, col0 : col0 + k], obj_d[s][:].rearrange("(p j) -> p j", p=128)
        )
        col0 += k
    nc.vector.tensor_scalar(OBJ[:], OBJ[:], -1.0, 1.0, op0=OP.mult, op1=OP.add)
    dacc = sg.tile([128, 1], F32)
    nc.scalar.activation(OBJ[:], OBJ[:], AF.Ln, bias=epsb[:, 0:1], scale=1.0,
                         accum_out=dacc[:])
    nc.vector.tensor_copy(SCAL[:, 6:7], dacc[:])

    # ---------------- partition reduce + output ----------------
    spsum = psum.tile([8, 1], F32)
    nc.tensor.matmul(spsum[:], lhsT=SCAL[:], rhs=ones128[:], start=True, stop=True)
    sout = sg.tile([8, 1], F32)
    nc.vector.tensor_copy(sout[:], spsum[:])
    nc.sync.dma_start(part_d[:], sout[:])

    ctx.close()


_PROG = None
TRACE = False
LAST_RESULTS = None


def _get_program():
    global _PROG
    if _PROG is None:
        _PROG = _build_program()
    return _PROG


def _host_constants():
    cst = np.zeros((NB, 3), np.float32)
    for b in range(BL):
        for s in range(3):
            cst[b * 3 + s, 0] = 1.0 / STRIDES[s]
            cst[b * 3 + s, 1] = WS[s]
            cst[b * 3 + s, 2] = b * A + BASES[s]
    sw = np.zeros((NCOL,), np.float32)
    for b in range(BL):
        for s in range(3):
            blk = b * 3 + s
            sw[blk * BLK : (blk + 1) * BLK] = STRIDES[s] / INPUT_W
    # winner-major vec weights: j = i*128 + p -> [p, i, c]
    wc4 = np.zeros((128, NW, 4), np.float32)
    for j in range(NCOL):
        p, i = j % 128, j // 128
        wc4[p, i, 0] = wc4[p, i, 1] = sw[j]
        wc4[p, i, 2] = wc4[p, i, 3] = 10.0
    return cst, wc4.reshape(128, NW * 4)


def kernel(input0, input1, input2, labels):
    nc = _get_program()
    cst, wc4 = _host_constants()
    # channels-last rows, all scales concatenated along the anchor dim
    ins = (input0, input1, input2)
    cl = np.concatenate(
        [
            np.asarray(t, np.float32)
            .transpose(0, 2, 3, 1)
            .reshape(B, -1, CH)
            for t in ins
        ],
        axis=1,
    )
    objs = [np.ascontiguousarray(np.asarray(t, np.float32)[:, 4]).reshape(B, -1)
            for t in ins]
    in_maps = []
    for c in range(NCORES):
        sl = slice(c * BL, (c + 1) * BL)
        m = {
            "cl": np.ascontiguousarray(cl[sl]),
            "labels": np.ascontiguousarray(np.asarray(labels, np.float32)[sl]),
            "cst": cst,
            "wc4": wc4,
        }
        for s in range(3):
            m[f"obj{s}"] = np.ascontiguousarray(objs[s][sl]).reshape(-1)
        in_maps.append(m)
    global LAST_RESULTS
    res = run_bass_kernel_spmd(
        nc, in_maps, core_ids=list(range(NCORES)), trace=TRACE
    )
    LAST_RESULTS = res
    P = np.stack([r["partials"] for r in res.results]).astype(np.float64)
    a_sum = P[:, 0:4].sum(axis=0)
    loss_obj = -(P[:, 4].sum() + P[:, 6].sum())
    loss_cls = -500.0 * P[:, 5].sum()
    loss_vec = a_sum.sum() * 10000.0
    loss = np.float32(loss_obj + loss_vec + loss_cls)
    return (np.array(loss, np.float32), a_sum.astype(np.float32))


if __name__ == "__main__":
    _get_program()
    print("program built OK")
